# revision 8
# baseline (speedup 1.0000x reference)
"""Trainium2 Bass kernel: row-GEMV + tanh-GELU + per-256-row-block max.

Computes, for x[65536, 2048], w[1, 2048], b[1]:
    y = x @ w[0] + b[0]
    p = y / 4
    s = p * (1 + tanh(0.7978845608 * (p + 0.044715 p^3)))   # == 2 * gelu(p)
    out = zeros(65536); out[256*i] = max(s[256*i : 256*i+256])

Sharding: x split row-wise across 8 NeuronCores (8192 rows each); w and b
replicated. Each core computes its 32 block maxima; the host scatters them
into the (mostly zero) full output.

Written in raw Bass (no Tile): this container's walrus build rejects any
instruction carrying more than one sync-wait command ("Too many sync wait
commands"), and Tile's semaphore assignment freely attaches several. In raw
Bass every wait is its own instruction, so each instruction carries at most
one wait.

Per-core pipeline (triple-buffered):
  SP:  stream 64 x-tiles of [128 rows, 2048] f32 from HBM (G tiles per DMA)
  DVE: per tile one fused scalar_tensor_tensor pass computes prod = x*w and
       accum_out = row-dot; then the exact tanh-GELU chain on the [128, 64]
       dot matrix and a pairwise column max ([128, 32])
  ACT: the tanh
  PE:  transpose [128, 32] -> PSUM [32, 128] (identity supplied as an input)
  DVE: free-dim max -> [32, 1] block maxima;  SP: DMA out (128 B)
"""

import os

import numpy as np

import concourse.bass as bass
from concourse import mybir
from concourse.bass_utils import run_bass_kernel_spmd

F32 = mybir.dt.float32

N_CORES = 8
BATCH = 65536
IN_F = 2048
BLOCK = 256
SHARD_ROWS = BATCH // N_CORES          # 8192
N_TILES = SHARD_ROWS // 128            # 64  (128-row tiles)
N_BLOCKS = SHARD_ROWS // BLOCK         # 32  (one output value each)
G = 2                                  # 128-row tiles per input DMA
N_ITER = N_TILES // G                  # 32 DMA iterations
NBUF = 3                               # x-tile buffers in flight

GELU_C = 0.7978845608
GELU_A = 0.044715
INV_POOL = 0.25


def _build() -> bass.Bass:
    nc = bass.Bass(trn_type="TRN2")
    x = nc.dram_tensor("x", [SHARD_ROWS, IN_F], F32, kind="ExternalInput")
    w = nc.dram_tensor("weight", [1, IN_F], F32, kind="ExternalInput")
    b = nc.dram_tensor("bias", [1, 1], F32, kind="ExternalInput")
    ident = nc.dram_tensor("ident", [128, 128], F32, kind="ExternalInput")
    out = nc.dram_tensor("out", [N_BLOCKS, 1], F32, kind="ExternalOutput")

    # [t, p, g, m]: row (t*G + g)*128 + p, feature m
    xv = x[:, :].rearrange("(t g p) m -> t p g m", g=G, p=128)

    mult = mybir.AluOpType.mult
    add = mybir.AluOpType.add
    amax = mybir.AluOpType.max

    from contextlib import ExitStack

    with ExitStack() as ctx:
        xt = ctx.enter_context(nc.sbuf_tensor("xt", [128, NBUF, G, IN_F], F32))
        wt = ctx.enter_context(nc.sbuf_tensor("wt", [128, IN_F], F32))
        bt = ctx.enter_context(nc.sbuf_tensor("bt", [128, 1], F32))
        idt = ctx.enter_context(nc.sbuf_tensor("idt", [128, 128], F32))
        prod = ctx.enter_context(nc.sbuf_tensor("prod", [128, IN_F], F32))
        y_all = ctx.enter_context(nc.sbuf_tensor("y_all", [128, N_TILES], F32))
        pp = ctx.enter_context(nc.sbuf_tensor("pp", [128, N_TILES], F32))
        p2 = ctx.enter_context(nc.sbuf_tensor("p2", [128, N_TILES], F32))
        qq = ctx.enter_context(nc.sbuf_tensor("qq", [128, N_TILES], F32))
        uu = ctx.enter_context(nc.sbuf_tensor("uu", [128, N_TILES], F32))
        th = ctx.enter_context(nc.sbuf_tensor("th", [128, N_TILES], F32))
        t1 = ctx.enter_context(nc.sbuf_tensor("t1", [128, N_TILES], F32))
        ss = ctx.enter_context(nc.sbuf_tensor("ss", [128, N_TILES], F32))
        sm = ctx.enter_context(nc.sbuf_tensor("sm", [128, N_BLOCKS], F32))
        pmax = ctx.enter_context(nc.sbuf_tensor("pmax", [N_BLOCKS, 1], F32))
        smt = ctx.enter_context(nc.psum_tensor("smt", [N_BLOCKS, 128], F32))
        # One DMA-completion semaphore per x-tile buffer slot: the free_sem
        # interlock ensures at most one in-flight DMA per slot, so a
        # threshold of 16*(reuse+1) is the slot's max possible count and
        # unambiguously means "that reuse fully landed". (A single shared
        # DMA semaphore is racy: 16 per-engine-slot +1 increments from
        # later in-flight DMAs can reach an earlier DMA's threshold while
        # it is still landing — observed as stale-tile reads under the
        # profiler's timing skew.)
        slot_sem = [
            ctx.enter_context(nc.semaphore(name=f"slot_sem{s}")) for s in range(NBUF)
        ]
        const_sem = ctx.enter_context(nc.semaphore())  # wt/bt/ident loads
        out_sem = ctx.enter_context(nc.semaphore())    # output DMA
        free_sem = ctx.enter_context(nc.semaphore())  # +1 per x-tile buf released
        dve_sem = ctx.enter_context(nc.semaphore())   # DVE milestones
        act_sem = ctx.enter_context(nc.semaphore())   # tanh done
        pe_sem = ctx.enter_context(nc.semaphore())    # transpose done
        block = ctx.enter_context(nc.Block())
        PROLOGUE = 48  # wt + bt + ident DMAs

        @block.sync
        def _(sync):
            sync.dma_start(wt[:, :], w[0:1, :].to_broadcast([128, IN_F])).then_inc(
                const_sem, 16
            )
            sync.dma_start(bt[:, :], b[0:1, :].to_broadcast([128, 1])).then_inc(
                const_sem, 16
            )
            sync.dma_start(idt[:, :], ident[:, :]).then_inc(const_sem, 16)
            for i in range(N_ITER):
                if i >= NBUF:
                    sync.wait_ge(free_sem, i - NBUF + 1)
                sync.dma_start(xt[:, i % NBUF, :, :], xv[i]).then_inc(
                    slot_sem[i % NBUF], 16
                )
            # output
            sync.wait_ge(dve_sem, 3)
            sync.dma_start(out[:, :], pmax[:, :]).then_inc(out_sem, 16)

        @block.vector
        def _(vector):
            vector.wait_ge(const_sem, PROLOGUE)
            for i in range(N_ITER):
                vector.wait_ge(slot_sem[i % NBUF], 16 * (i // NBUF + 1))
                for g in range(G):
                    t = i * G + g
                    ins = nc.vector.scalar_tensor_tensor(
                        out=prod[:, :],
                        in0=xt[:, i % NBUF, g, :],
                        scalar=1.0,
                        in1=wt[:, :],
                        op0=mult,
                        op1=mult,
                        accum_out=y_all[:, t : t + 1],
                    )
                    if g == G - 1:
                        ins.then_inc(free_sem, 1)
            # p = (y + b) / 4
            nc.vector.tensor_scalar(
                pp[:, :], y_all[:, :], bt[:, 0:1], INV_POOL, op0=add, op1=mult
            )
            nc.vector.tensor_mul(p2[:, :], pp[:, :], pp[:, :])
            nc.vector.tensor_scalar(
                qq[:, :], p2[:, :], GELU_A, 1.0, op0=mult, op1=add
            )
            nc.vector.tensor_mul(uu[:, :], pp[:, :], qq[:, :]).then_inc(dve_sem, 1)
            # ACT computes th = tanh(C*uu) here
            vector.wait_ge(act_sem, 1)
            nc.vector.tensor_scalar_add(t1[:, :], th[:, :], 1.0)
            nc.vector.tensor_mul(ss[:, :], t1[:, :], pp[:, :])
            nc.vector.tensor_reduce(
                sm[:, :],
                ss[:, :].rearrange("p (b two) -> p b two", two=2),
                axis=mybir.AxisListType.X,
                op=amax,
            ).then_inc(dve_sem, 1)
            # PE transposes sm into PSUM here
            vector.wait_ge(pe_sem, 1)
            nc.vector.tensor_reduce(
                pmax[:, :], smt[:, :], axis=mybir.AxisListType.X, op=amax
            ).then_inc(dve_sem, 1)

        @block.scalar
        def _(scalar):
            scalar.wait_ge(dve_sem, 1)
            nc.scalar.activation(
                th[:, :], uu[:, :], mybir.ActivationFunctionType.Tanh, scale=GELU_C
            ).then_inc(act_sem, 1)

        @block.tensor
        def _(tensor):
            tensor.wait_ge(const_sem, PROLOGUE)  # identity loaded
            tensor.wait_ge(dve_sem, 2)           # sm ready
            nc.tensor.transpose(smt[:, :], sm[:, :], idt[:, :]).then_inc(pe_sem, 1)

    return nc


_CACHE: dict = {}
LAST_RESULT = None  # BassKernelResults from the most recent kernel() call


def _get_nc() -> bass.Bass:
    if "nc" not in _CACHE:
        _CACHE["nc"] = _build()
    return _CACHE["nc"]


def kernel(x, weight, bias, **run_kwargs) -> np.ndarray:
    global LAST_RESULT
    x = np.ascontiguousarray(np.asarray(x, dtype=np.float32))
    weight = np.ascontiguousarray(np.asarray(weight, dtype=np.float32)).reshape(1, IN_F)
    bias = np.ascontiguousarray(np.asarray(bias, dtype=np.float32)).reshape(1, 1)
    assert x.shape == (BATCH, IN_F)
    ident = np.eye(128, dtype=np.float32)

    nc = _get_nc()
    in_maps = [
        {
            "x": x[c * SHARD_ROWS : (c + 1) * SHARD_ROWS],
            "weight": weight,
            "bias": bias,
            "ident": ident,
        }
        for c in range(N_CORES)
    ]
    res = run_bass_kernel_spmd(nc, in_maps, core_ids=list(range(N_CORES)), **run_kwargs)
    LAST_RESULT = res

    out = np.zeros(BATCH, dtype=np.float32)
    idx = np.arange(N_BLOCKS) * BLOCK
    for c in range(N_CORES):
        out[c * SHARD_ROWS + idx] = np.asarray(res.results[c]["out"]).reshape(N_BLOCKS)
    return out


# revision 11
# speedup vs baseline: 1.0053x; 1.0053x over previous
"""Trainium2 Bass kernel: row-GEMV + tanh-GELU + per-256-row-block max.

Computes, for x[65536, 2048], w[1, 2048], b[1]:
    y = x @ w[0] + b[0]
    p = y / 4
    s = p * (1 + tanh(0.7978845608 * (p + 0.044715 p^3)))   # == 2 * gelu(p)
    out = zeros(65536); out[256*i] = max(s[256*i : 256*i+256])

Sharding: x split row-wise across 8 NeuronCores (8192 rows each); w and b
replicated. Each core computes its 32 block maxima; the host scatters them
into the (mostly zero) full output.

Written in raw Bass (no Tile): this container's walrus build rejects any
instruction carrying more than one sync-wait command ("Too many sync wait
commands"), and Tile's semaphore assignment freely attaches several. In raw
Bass every wait is its own instruction.

Per-core pipeline (memory-bound; HBM floor ~64 MB / 358 GB/s = 187 us):
  SP+ACT: stream 32 x-DMAs of [128, 2, 2048] f32, alternating between the
       two HWDGE rings (qSPDynamicHW / qActDynamicHW) to overlap completion
       latencies; 4 buffer slots. The w/b/identity prologue goes on the ACT
       ring so the first x tile starts at t=0 on the SP ring.
  DVE: per 128-row tile one fused scalar_tensor_tensor computes prod = x*w
       (discarded, stride-0 dummy out) with accum_out = row dots; then the
       exact tanh-GELU chain on the [128, 64] dot matrix and a pairwise
       column max ([128, 32]).
  ACT: the tanh (table preloaded at t=0 by a dummy activation).
  PE:  transpose [128, 32] -> PSUM [32, 128] (identity is a kernel input).
  DVE: free-dim max -> [32, 1] block maxima;  SP: DMA out (128 B).

Sync protocol: one DMA-completion semaphore per x buffer slot. The
free_sem interlock guarantees at most one in-flight DMA per slot, so the
slot threshold 16*(reuse+1) is that slot's maximum possible count and
unambiguously means "fully landed". (A single shared DMA semaphore is racy:
the 16 per-engine +1 increments of later in-flight DMAs can reach an
earlier DMA's threshold while it is still landing — observed as stale-tile
reads under profiler timing skew.)
"""

from contextlib import ExitStack

import numpy as np

import concourse.bass as bass
from concourse import mybir
from concourse.bass_utils import run_bass_kernel_spmd

F32 = mybir.dt.float32

N_CORES = 8
BATCH = 65536
IN_F = 2048
BLOCK = 256
SHARD_ROWS = BATCH // N_CORES          # 8192
N_TILES = SHARD_ROWS // 128            # 64  (128-row tiles)
N_BLOCKS = SHARD_ROWS // BLOCK         # 32  (one output value each)
G = 2                                  # 128-row tiles per input DMA
N_ITER = N_TILES // G                  # 32 DMA iterations
NBUF = 4                               # x-tile buffer slots

GELU_C = 0.7978845608
GELU_A = 0.044715
INV_POOL = 0.25


def _build() -> bass.Bass:
    nc = bass.Bass(trn_type="TRN2")
    x = nc.dram_tensor("x", [SHARD_ROWS, IN_F], F32, kind="ExternalInput")
    w = nc.dram_tensor("weight", [1, IN_F], F32, kind="ExternalInput")
    b = nc.dram_tensor("bias", [1, 1], F32, kind="ExternalInput")
    ident = nc.dram_tensor("ident", [128, 128], F32, kind="ExternalInput")
    out = nc.dram_tensor("out", [N_BLOCKS, 1], F32, kind="ExternalOutput")

    # [t, p, g, m]: row (t*G + g)*128 + p, feature m
    xv = x[:, :].rearrange("(t g p) m -> t p g m", g=G, p=128)

    mult = mybir.AluOpType.mult
    add = mybir.AluOpType.add
    amax = mybir.AluOpType.max

    with ExitStack() as ctx:
        xt = ctx.enter_context(nc.sbuf_tensor("xt", [128, NBUF, G, IN_F], F32))
        wt = ctx.enter_context(nc.sbuf_tensor("wt", [128, IN_F], F32))
        bt = ctx.enter_context(nc.sbuf_tensor("bt", [128, 1], F32))
        idt = ctx.enter_context(nc.sbuf_tensor("idt", [128, 128], F32))
        dummy = ctx.enter_context(nc.sbuf_tensor("stt_dump", [128, 1], F32))
        actw = ctx.enter_context(nc.sbuf_tensor("actw", [1, 1], F32))
        y_all = ctx.enter_context(nc.sbuf_tensor("y_all", [128, N_TILES], F32))
        pp = ctx.enter_context(nc.sbuf_tensor("pp", [128, N_TILES], F32))
        p2 = ctx.enter_context(nc.sbuf_tensor("p2", [128, N_TILES], F32))
        qq = ctx.enter_context(nc.sbuf_tensor("qq", [128, N_TILES], F32))
        uu = ctx.enter_context(nc.sbuf_tensor("uu", [128, N_TILES], F32))
        th = ctx.enter_context(nc.sbuf_tensor("th", [128, N_TILES], F32))
        t1 = ctx.enter_context(nc.sbuf_tensor("t1", [128, N_TILES], F32))
        ss = ctx.enter_context(nc.sbuf_tensor("ss", [128, N_TILES], F32))
        sm = ctx.enter_context(nc.sbuf_tensor("sm", [128, N_BLOCKS], F32))
        pmax = ctx.enter_context(nc.sbuf_tensor("pmax", [N_BLOCKS, 1], F32))
        smt = ctx.enter_context(nc.psum_tensor("smt", [N_BLOCKS, 128], F32))
        slot_sem = [
            ctx.enter_context(nc.semaphore(name=f"slot_sem{s}")) for s in range(NBUF)
        ]
        wt_sem = ctx.enter_context(nc.semaphore())     # weight load
        const_sem = ctx.enter_context(nc.semaphore())  # bt/ident loads
        out_sem = ctx.enter_context(nc.semaphore())    # output DMA
        free_sem = ctx.enter_context(nc.semaphore())   # +1 per x slot released
        dve_sem = ctx.enter_context(nc.semaphore())    # DVE milestones
        act_sem = ctx.enter_context(nc.semaphore())    # tanh done
        pe_sem = ctx.enter_context(nc.semaphore())     # transpose done
        block = ctx.enter_context(nc.Block())


        def issue_x_dmas(eng, parity):
            for i in range(N_ITER):
                if i % 2 != parity:
                    continue
                if i >= NBUF:
                    eng.wait_ge(free_sem, i - NBUF + 1)
                eng.dma_start(xt[:, i % NBUF, :, :], xv[i]).then_inc(
                    slot_sem[i % NBUF], 16
                )

        @block.sync
        def _(sync):
            issue_x_dmas(sync, 0)
            sync.wait_ge(dve_sem, 3)
            sync.dma_start(out[:, :], pmax[:, :]).then_inc(out_sem, 16)

        @block.scalar
        def _(scalar):
            scalar.dma_start(wt[:, :], w[0:1, :].to_broadcast([128, IN_F])).then_inc(
                wt_sem, 16
            )
            scalar.dma_start(bt[:, :], b[0:1, :].to_broadcast([128, 1])).then_inc(
                const_sem, 16
            )
            scalar.dma_start(idt[:, :], ident[:, :]).then_inc(const_sem, 16)
            # Preload the tanh spline tables while the stream runs.
            nc.scalar.activation(
                actw[:, :], actw[:, :], mybir.ActivationFunctionType.Tanh
            )
            issue_x_dmas(scalar, 1)
            # the real tanh of the gelu chain
            scalar.wait_ge(dve_sem, 1)
            nc.scalar.activation(
                th[:, :], uu[:, :], mybir.ActivationFunctionType.Tanh, scale=GELU_C
            ).then_inc(act_sem, 1)

        @block.vector
        def _(vector):
            vector.wait_ge(wt_sem, 16)  # wt loaded
            for i in range(N_ITER):
                vector.wait_ge(slot_sem[i % NBUF], 16 * (i // NBUF + 1))
                for g in range(G):
                    t = i * G + g
                    ins = nc.vector.scalar_tensor_tensor(
                        out=dummy[:, :].broadcast_to((128, IN_F)),
                        in0=xt[:, i % NBUF, g, :],
                        scalar=1.0,
                        in1=wt[:, :],
                        op0=mult,
                        op1=mult,
                        accum_out=y_all[:, t : t + 1],
                    )
                    if g == G - 1:
                        ins.then_inc(free_sem, 1)
            # p = (y + b) / 4
            vector.wait_ge(const_sem, 32)  # bt+ident loaded (max count)
            nc.vector.tensor_scalar(
                pp[:, :], y_all[:, :], bt[:, 0:1], INV_POOL, op0=add, op1=mult
            )
            nc.vector.tensor_mul(p2[:, :], pp[:, :], pp[:, :])
            nc.vector.tensor_scalar(
                qq[:, :], p2[:, :], GELU_A, 1.0, op0=mult, op1=add
            )
            nc.vector.tensor_mul(uu[:, :], pp[:, :], qq[:, :]).then_inc(dve_sem, 1)
            # ACT computes th = tanh(C*uu) here
            vector.wait_ge(act_sem, 1)
            nc.vector.tensor_scalar_add(t1[:, :], th[:, :], 1.0)
            nc.vector.tensor_mul(ss[:, :], t1[:, :], pp[:, :])
            nc.vector.tensor_reduce(
                sm[:, :],
                ss[:, :].rearrange("p (b two) -> p b two", two=2),
                axis=mybir.AxisListType.X,
                op=amax,
            ).then_inc(dve_sem, 1)
            # PE transposes sm into PSUM here
            vector.wait_ge(pe_sem, 1)
            nc.vector.tensor_reduce(
                pmax[:, :], smt[:, :], axis=mybir.AxisListType.X, op=amax
            ).then_inc(dve_sem, 1)

        @block.tensor
        def _(tensor):
            tensor.wait_ge(const_sem, 32)  # bt+ident loaded (max count)
            tensor.wait_ge(dve_sem, 2)           # sm ready
            nc.tensor.transpose(smt[:, :], sm[:, :], idt[:, :]).then_inc(pe_sem, 1)

    return nc


_CACHE: dict = {}
LAST_RESULT = None  # BassKernelResults from the most recent kernel() call


def _get_nc() -> bass.Bass:
    if "nc" not in _CACHE:
        _CACHE["nc"] = _build()
    return _CACHE["nc"]


def kernel(x, weight, bias, **run_kwargs) -> np.ndarray:
    global LAST_RESULT
    x = np.ascontiguousarray(np.asarray(x, dtype=np.float32))
    weight = np.ascontiguousarray(np.asarray(weight, dtype=np.float32)).reshape(1, IN_F)
    bias = np.ascontiguousarray(np.asarray(bias, dtype=np.float32)).reshape(1, 1)
    assert x.shape == (BATCH, IN_F)
    ident = np.eye(128, dtype=np.float32)

    nc = _get_nc()
    in_maps = [
        {
            "x": x[c * SHARD_ROWS : (c + 1) * SHARD_ROWS],
            "weight": weight,
            "bias": bias,
            "ident": ident,
        }
        for c in range(N_CORES)
    ]
    res = run_bass_kernel_spmd(nc, in_maps, core_ids=list(range(N_CORES)), **run_kwargs)
    LAST_RESULT = res

    out = np.zeros(BATCH, dtype=np.float32)
    idx = np.arange(N_BLOCKS) * BLOCK
    for c in range(N_CORES):
        out[c * SHARD_ROWS + idx] = np.asarray(res.results[c]["out"]).reshape(N_BLOCKS)
    return out
